# revision 20
# baseline (speedup 1.0000x reference)
"""Trainium2 Bass kernel for nn_BBoxHeadForGroundTruthBboxRegressionV1.

Strategy (v3: fp8 DoubleRow, pipelined DMA stream)
--------------------------------------------------
Per packed token t (T=2048):
    feat[t] = concat(vision_flat[idx[t]], grd_tokens[t])    # [25600]
    out = mlp5(feat)                                        # 25600->1024^4->6

Algebraic restructure (as v1): the first-layer matmul commutes with the row
gather, so  feat @ w0 = (vision_flat @ w0_v)[idx] + grd_tokens @ w0_lm,
with the tiny [8, 1024] P matrix computed on host and gathered on-device via
an exact fp32r one-hot matmul.

All heavy matmuls are fp8 with perf_mode=DoubleRow (contraction 256 per
instruction, half-rate per output column):
  - Layer 0 (grd @ w0_lm): both operands naive e4m3 (the exact fp32 vision
    path keeps the total error at ~1.7%).
  - Layers 1-3: weights and activations split hi(e4m3) + lo(e5m2) at the
    SAME scale, z = h_hi @ (w_hi + w_lo) + h_lo @ w_hi: three half-rate
    matmuls replace two bf16 ones at near-bf16 accuracy, sharing one PSUM
    accumulation chain.
  - Layer 4 (1024->6): plain bf16 on the bf16 relu output; the 1/SW scale
    is folded into w4 so layer-3 relu needs no scale and can be split
    across the Scalar and Vector engines.
The kernel is DMA-bound (~11 MB/core at 360 GB/s): the DMA stream is
ordered in exact consumption order (grd path first, then per-layer hi
weights before lo), activations are produced in 4 pair-tiles per layer so
consumers wait at [128,2,256] granularity, and dummy fp32r matmuls keep
the PE busy ramp alive while layer 0 is DMA-paced.
"""

import ml_dtypes
import numpy as np

import concourse.bass as bass
import concourse.tile as tile
from concourse import bacc, mybir
from concourse.bass import ts
from concourse.bass_utils import run_bass_kernel_spmd

B, L, T, LM, DFF, D, H = 8, 256, 2048, 4096, 1024, 84, 4
HD = D // H
NCLS = 265
VF = D * L  # 21504 vision features per sample
NCORES = 8
TPC = T // NCORES  # 256 tokens per core
C0 = LM // 256  # 16 DoubleRow chunks for the grd matmul
C1 = DFF // 256  # 4 DoubleRow chunks for the hidden layers
JB = DFF // 128  # 8 output blocks of 128 features

S0 = 2.0 ** 12  # w0lm fp8 scale
SW = 2.0 ** 10  # w1..w3 fp8 scale

FILL_PRE = 29    # warm-up filler matmuls between one-hot and layer 0
FILL_CHUNK = 3   # fillers per layer-0 chunk (PE keeps ramp while DMA-paced)
FILL_B = 7      # fillers straddling the layer-0 -> h0-split boundary

F32 = mybir.dt.float32
F32R = mybir.dt.float32r
BF16 = mybir.dt.bfloat16
F8E4 = mybir.dt.float8e4
F8E5 = mybir.dt.float8e5
NPBF16 = ml_dtypes.bfloat16
NP8E4 = ml_dtypes.float8_e4m3
NP8E5 = ml_dtypes.float8_e5m2
RELU = mybir.ActivationFunctionType.Relu
IDENT = mybir.ActivationFunctionType.Identity
DR = mybir.MatmulPerfMode.DoubleRow
MUL = mybir.AluOpType.mult
MAX = mybir.AluOpType.max
COPY = mybir.ActivationFunctionType.Copy

_CACHE = {}


def _build_bass():
    nc = bacc.Bacc(
        "TRN2", target_bir_lowering=False, debug=False, num_devices=NCORES
    )
    inp = {}
    inp["poh"] = nc.dram_tensor("poh", [B, DFF + TPC], F32, kind="ExternalInput")
    inp["grdT"] = nc.dram_tensor("grdT", [128, C0, 2, TPC], F8E4, kind="ExternalInput")
    inp["w0lm"] = nc.dram_tensor("w0lm", [128, C0, 2, DFF], F8E4, kind="ExternalInput")
    for w in ("w1", "w2", "w3"):
        inp[w] = nc.dram_tensor(w, [128, 2, C1, 2, DFF], F8E4, kind="ExternalInput")
    inp["w4"] = nc.dram_tensor("w4", [128, JB, 6], BF16, kind="ExternalInput")
    # biasrow: [b1, b2, b3]*SW as row vectors for in-chain bias matmuls
    inp["biasrow"] = nc.dram_tensor("biasrow", [B, 3 * DFF], F32, kind="ExternalInput")
    inp["b4"] = nc.dram_tensor("b4", [6, 1], F32, kind="ExternalInput")
    out = nc.dram_tensor("out", [6, TPC], F32, kind="ExternalOutput")

    with tile.TileContext(nc) as tc:
        with (
            tc.tile_pool(name="big", bufs=1) as big,
            tc.tile_pool(name="act", bufs=2) as actp,
            tc.tile_pool(name="psum", bufs=8, space="PSUM") as pp,
            tc.tile_pool(name="outp", bufs=1) as outp,
        ):
            # --- static tiles -----------------------------------------------
            poh_sb = big.tile([B, DFF + TPC], F32R)
            paug_sb = poh_sb[:, :DFF]
            oh_sb = poh_sb[:, DFF:]
            grdT_sb = big.tile([128, C0, 2, TPC], F8E4)
            w0_sb = big.tile([128, C0, 2, DFF], F8E4)
            w_sb = {}
            for w in ("w1", "w2", "w3"):
                w_sb[w] = big.tile(
                    [128, 2, C1, 2, DFF], F8E4, name=f"{w}_sb", tag=f"{w}_sb"
                )
            w4_sb = big.tile([128, JB, 6], BF16)
            biasrow_sb = big.tile([B, 3 * DFF], F32R)
            b4_sb = big.tile([6, 1], F32)

            # --- DMA stream (issue order == consumption order) --------------
            nc.sync.dma_start(poh_sb[:], inp["poh"][:].bitcast(F32R))
            nc.sync.dma_start(grdT_sb[:], inp["grdT"][:])
            nc.sync.dma_start(biasrow_sb[:], inp["biasrow"][:].bitcast(F32R))
            for g in range(C0):
                nc.sync.dma_start(w0_sb[:, g : g + 1], inp["w0lm"][:, g : g + 1])
            for w in ("w1", "w2"):
                nc.sync.dma_start(w_sb[w][:, 0], inp[w][:, 0])
                nc.sync.dma_start(w_sb[w][:, 1], inp[w][:, 1])
            # w3 streams last: hi, then lo in two pieces so only 8 small
            # matmuls trail the final bytes.
            nc.sync.dma_start(w_sb["w3"][:, 0], inp["w3"][:, 0])
            nc.sync.dma_start(w_sb["w3"][:, 1, 0:3], inp["w3"][:, 1, 0:3])
            nc.sync.dma_start(w_sb["w3"][:, 1, 3:4], inp["w3"][:, 1, 3:4])
            nc.sync.dma_start(b4_sb[:], inp["b4"][:])
            nc.sync.dma_start(w4_sb[:], inp["w4"][:])

            def whi(w, c, jb):
                return w_sb[w][:, 0, c, :, ts(jb, 128)]

            def wlo(w, c, jb):
                return w_sb[w][:, 1, c, :, ts(jb, 128)].bitcast(F8E5)

            # Zero stationary operand for warm-up fillers: accumulating
            # 0 @ x into a live chain is a numeric no-op that keeps PE busy.
            zt = big.tile([B, 128], F32)
            nc.vector.memset(zt[:], 0.0)

            # --- layer 0: z0 = P_pick + grdT.T @ w0lm (both fp8 e4m3) -------
            ps = [
                pp.tile([128, TPC], F32, tag="ps", name=f"ps0_{j}") for j in range(JB)
            ]

            def fill(n):
                # 64-col fp32r dummies (~107ns at any p-state) accumulated as
                # zeros into chain 7; keeps the PE ramp alive while layer 0
                # is DMA-paced.
                for i in range(n):
                    nc.tensor.matmul(
                        ps[7][:, 0:64],
                        lhsT=zt[:].bitcast(F32R),
                        rhs=oh_sb[:, 0:64],
                        start=False,
                        stop=False,
                    )
            for jb in range(JB):
                nc.tensor.matmul(
                    ps[jb][:],
                    lhsT=paug_sb[:, ts(jb, 128)],
                    rhs=oh_sb[:],
                    start=True,
                    stop=False,
                )
            fill(FILL_PRE)
            for c in range(C0):
                last = c == C0 - 1
                for jb in range(JB):
                    if last and jb == 7:
                        fill(FILL_B)
                    nc.tensor.matmul(
                        ps[jb][:],
                        lhsT=w0_sb[:, c, :, ts(jb, 128)],
                        rhs=grdT_sb[:, c],
                        start=False,
                        stop=last,
                        perf_mode=DR,
                    )
                if c < 14:
                    fill(FILL_CHUNK)

            def htiles(layer):
                t = [actp.tile([128, 2, TPC], BF16, tag=f"t{p}", name=f"t{layer}_{p}")
                     for p in range(4)]
                hhi = [actp.tile([128, 2, TPC], F8E4, tag=f"hhi{p}",
                                 name=f"hhi{layer}_{p}") for p in range(4)]
                hlo = [actp.tile([128, 2, TPC], F8E5, tag=f"hlo{p}",
                                 name=f"hlo{layer}_{p}") for p in range(4)]
                return t, hhi, hlo

            def boundary(ps, t, hhi, hlo, scale):
                """psum -> hhi(e4m3) + t(bf16) + hlo(e5m2 = t - hhi).
                hhi comes straight from PSUM (no copy); work is split so the
                hhi pairs the next layer consumes first are ready earliest:
                ACT: hhi even blocks, then t odd; DVE: hhi odd, t even, subs."""
                with tc.high_priority():
                    for p in range(4):
                        nc.scalar.activation(
                            hhi[p][:, 0], ps[2 * p][:], RELU, scale=scale)
                    for p in range(4):
                        nc.vector.tensor_scalar(
                            hhi[p][:, 1], ps[2 * p + 1][:], scale, 0.0, MUL, MAX)
                for p in range(4):
                    nc.scalar.activation(
                        t[p][:, 1], ps[2 * p + 1][:], RELU, scale=scale)
                for p in range(4):
                    nc.vector.tensor_scalar(
                        t[p][:, 0], ps[2 * p][:], scale, 0.0, MUL, MAX)
                for jb in range(JB):
                    nc.vector.tensor_sub(
                        hlo[jb // 2][:, jb % 2], t[jb // 2][:, jb % 2],
                        hhi[jb // 2][:, jb % 2],
                    )

            t0, h0hi, h0lo = htiles(0)
            boundary(ps, t0, h0hi, h0lo, 1.0 / S0)

            # --- layers 1..3: z = h_hi@(w_hi+w_lo) + h_lo@w_hi + b ----------
            # hi-term groups c-major (consume h pairs as they appear); the
            # lo-weight group jb-major so chains stop staggered; chain ends
            # with the fp32r bias matmul (b*SW broadcast via ones vector).
            hhi, hlo = h0hi, h0lo
            for li, w in ((1, "w1"), (2, "w2"), (3, "w3")):
                ps = [
                    pp.tile([128, TPC], F32, tag="ps", name=f"ps{li}_{j}")
                    for j in range(JB)
                ]
                # chains start with the bias matmuls: no h/w dependency, so
                # they run during the w-hi DMA wait.
                for jb in range(JB):
                    nc.tensor.matmul(
                        ps[jb][:],
                        lhsT=biasrow_sb[:, ts(8 * (li - 1) + jb, 128)],
                        rhs=oh_sb[:],
                        start=True, stop=False,
                    )
                if li == 1:
                    # cover the tail of the w1-hi DMA wait
                    for i in range(4):
                        nc.tensor.matmul(
                            ps[7][:, 0:64], lhsT=zt[:].bitcast(F32R),
                            rhs=oh_sb[:, 0:64], start=False, stop=False,
                        )
                for c in range(C1):
                    for jb in range(JB):
                        nc.tensor.matmul(
                            ps[jb][:], lhsT=whi(w, c, jb), rhs=hhi[c][:],
                            start=False, stop=False, perf_mode=DR,
                        )
                # per-jb subchains: hi(lo-act) then lo-weight terms; chain
                # stops stagger across ~3.4us so the ACT/DVE boundary ops for
                # the next layer keep pace with the PE stream.
                for jb in range(JB):
                    for c in range(C1):
                        nc.tensor.matmul(
                            ps[jb][:], lhsT=whi(w, c, jb), rhs=hlo[c][:],
                            start=False, stop=False, perf_mode=DR,
                        )
                    for c in range(C1):
                        nc.tensor.matmul(
                            ps[jb][:], lhsT=wlo(w, c, jb), rhs=hhi[c][:],
                            start=False, stop=(c == C1 - 1), perf_mode=DR,
                        )
                if li < 3:
                    t, nhi, nlo = htiles(li)
                    boundary(ps, t, nhi, nlo, 1.0 / SW)
                    hhi, hlo = nhi, nlo

            # --- layer 3 relu (scale folded into w4) + layer 4 --------------
            t3 = [actp.tile([128, 2, TPC], BF16, tag=f"t{p}", name=f"t3_{p}")
                  for p in range(4)]
            HT = TPC // 2
            ps4a = pp.tile([128, HT], F32, tag="ps", name="ps4a")[:6]
            ps4b = pp.tile([128, HT], F32, tag="ps", name="ps4b")[:6]
            for jb in range(JB):
                dst = t3[jb // 2][:, jb % 2]
                if jb % 2 == 0:
                    nc.scalar.activation(dst, ps[jb][:], RELU)
                else:
                    nc.vector.tensor_scalar_max(dst, ps[jb][:], 0.0)
                nc.tensor.matmul(
                    ps4a[:], lhsT=w4_sb[:, jb], rhs=dst[:, 0:HT],
                    start=(jb == 0), stop=(jb == JB - 1),
                )
                nc.tensor.matmul(
                    ps4b[:], lhsT=w4_sb[:, jb], rhs=dst[:, HT:],
                    start=(jb == 0), stop=(jb == JB - 1),
                )
            out_sb = outp.tile([6, TPC], F32)
            nc.scalar.activation(out_sb[:, 0:HT], ps4a[:], IDENT, bias=b4_sb[:, 0:1])
            nc.sync.dma_start(out[:, 0:HT], out_sb[:, 0:HT])
            nc.scalar.activation(out_sb[:, HT:], ps4b[:], IDENT, bias=b4_sb[:, 0:1])
            nc.sync.dma_start(out[:, HT:], out_sb[:, HT:])

    nc.compile()
    return nc


def _layernorm(x, s, b):
    m = x.mean(-1, keepdims=True)
    v = ((x - m) ** 2).mean(-1, keepdims=True)
    return (x - m) / np.sqrt(v + np.float32(1e-5)) * s + b


def _host_encoder(vision_features, gauss_B, class_emb, w_in, b_in, w_out, b_out,
                  ln1_s, ln1_b, w_ff1, b_ff1, w_ff2, b_ff2, ln2_s, ln2_b):
    """Numpy fp32 replica of the reference's tiny 2-layer encoder (~2% of FLOPs)."""
    two_pi = np.float32(2.0 * np.pi)

    def fourier(xyz):
        proj = two_pi * (xyz @ gauss_B)
        return np.concatenate([np.sin(proj), np.cos(proj)], axis=-1)

    cls = vision_features[:, :, -1].astype(np.int32)
    cls = np.clip(cls, 0, NCLS - 1)  # match jax's clamped gather
    src = np.concatenate(
        [fourier(vision_features[:, :, 0:3]),
         fourier(vision_features[:, :, 3:6]),
         class_emb[cls]],
        axis=-1,
    ).astype(np.float32)  # [B, L, 84]
    pad = np.all(vision_features == 0, axis=-1)
    neg = np.where(pad, np.float32(-1e9), np.float32(0.0))[:, None, None, :]
    inv_sqrt_hd = np.float32(1.0 / np.sqrt(HD))
    for lyr in range(2):
        qkv = src @ w_in[lyr] + b_in[lyr]
        q, k, v = np.split(qkv, 3, axis=-1)
        q = q.reshape(B, L, H, HD)
        k = k.reshape(B, L, H, HD)
        v = v.reshape(B, L, H, HD)
        scores = np.einsum("blhd,bmhd->bhlm", q, k) * inv_sqrt_hd + neg
        scores = scores - scores.max(-1, keepdims=True)
        e = np.exp(scores)
        attn = e / e.sum(-1, keepdims=True)
        o = np.einsum("bhlm,bmhd->blhd", attn, v).reshape(B, L, D)
        src = _layernorm(src + o @ w_out[lyr] + b_out[lyr], ln1_s[lyr], ln1_b[lyr])
        ff = np.maximum(src @ w_ff1[lyr] + b_ff1[lyr], 0) @ w_ff2[lyr] + b_ff2[lyr]
        src = _layernorm(src + ff, ln2_s[lyr], ln2_b[lyr])
    return src.reshape(B, L * D)  # [8, 21504]


def _q8(x):
    """fp8 e4m3 (TRN variant; clip to the +-240 finite range)."""
    return np.clip(x, -240.0, 240.0).astype(NP8E4)


def _q5(x):
    return np.clip(x, -57344.0, 57344.0).astype(NP8E5)


def _dr_arrange(x, nchunk, width):
    """[256*nchunk, width] -> [128, nchunk, 2, width] DoubleRow layout."""
    return np.ascontiguousarray(
        x.reshape(nchunk, 2, 128, width).transpose(2, 0, 1, 3)
    )


def kernel(grd_tokens, vision_features, token_batch_idx, gauss_B, class_emb,
           w_in, b_in, w_out, b_out, ln1_s, ln1_b, w_ff1, b_ff1, w_ff2, b_ff2,
           ln2_s, ln2_b, w0, b0, w1, b1, w2, b2, w3, b3, w4, b4,
           _trace=False):
    f32 = np.float32
    grd_tokens = np.asarray(grd_tokens, f32)
    vision_features = np.asarray(vision_features, f32)
    idx = np.asarray(token_batch_idx).astype(np.int64)
    w0 = np.asarray(w0, f32)
    b0 = np.asarray(b0, f32)

    # Vision branch on host (input marshalling + ~2.3 GF): encoder -> P matrix.
    vision_flat = _host_encoder(
        vision_features, np.asarray(gauss_B, f32), np.asarray(class_emb, f32),
        np.asarray(w_in, f32), np.asarray(b_in, f32), np.asarray(w_out, f32),
        np.asarray(b_out, f32), np.asarray(ln1_s, f32), np.asarray(ln1_b, f32),
        np.asarray(w_ff1, f32), np.asarray(b_ff1, f32), np.asarray(w_ff2, f32),
        np.asarray(b_ff2, f32), np.asarray(ln2_s, f32), np.asarray(ln2_b, f32),
    )
    paug = ((vision_flat @ w0[:VF] + b0) * f32(S0)).astype(f32)  # [8, 1024]

    # Shared (replicated) device inputs.
    shared = {"w0lm": _dr_arrange(_q8(w0[VF:] * f32(S0)), C0, DFF)}
    for name, w in (("w1", w1), ("w2", w2), ("w3", w3)):
        ws = np.asarray(w, f32) * f32(SW)
        hi = _q8(ws)
        lo = _q5(ws - hi.astype(f32))
        shared[name] = np.ascontiguousarray(
            np.stack(
                [
                    _dr_arrange(hi, C1, DFF),
                    _dr_arrange(lo.view(np.uint8).view(NP8E4), C1, DFF),
                ],
                axis=1,
            )
        )  # [128, 2, C1, 2, DFF] e4m3 container
    shared["w4"] = np.ascontiguousarray(
        (np.asarray(w4, f32) * f32(1.0 / SW))
        .reshape(JB, 128, 6).transpose(1, 0, 2).astype(NPBF16)
    )
    brow = np.concatenate([np.asarray(b, f32) for b in (b1, b2, b3)]) * f32(SW)
    shared["biasrow"] = np.ascontiguousarray(np.broadcast_to(brow, (B, 3 * DFF)))
    shared["b4"] = np.ascontiguousarray(np.asarray(b4, f32).reshape(6, 1))

    # Per-core shards.
    in_maps = []
    for m in range(NCORES):
        rows = slice(m * TPC, (m + 1) * TPC)
        grdT = _dr_arrange(_q8(grd_tokens[rows].T), C0, TPC)
        oh = (idx[rows][None, :] == np.arange(B)[:, None]).astype(f32)
        im = dict(shared)
        im["grdT"] = grdT
        im["poh"] = np.ascontiguousarray(np.concatenate([paug, oh], axis=1))
        in_maps.append(im)

    if "nc" not in _CACHE:
        _CACHE["nc"] = _build_bass()
    res = run_bass_kernel_spmd(
        _CACHE["nc"], in_maps, core_ids=list(range(NCORES)), trace=_trace
    )
    _CACHE["last_result"] = res
    out = np.concatenate([r["out"].T for r in res.results], axis=0)
    return np.ascontiguousarray(out.astype(f32))


# revision 42
# speedup vs baseline: 1.0159x; 1.0159x over previous
"""Trainium2 Bass kernel for nn_BBoxHeadForGroundTruthBboxRegressionV1.

Strategy (v3: fp8 DoubleRow, pipelined DMA stream)
--------------------------------------------------
Per packed token t (T=2048):
    feat[t] = concat(vision_flat[idx[t]], grd_tokens[t])    # [25600]
    out = mlp5(feat)                                        # 25600->1024^4->6

Algebraic restructure (as v1): the first-layer matmul commutes with the row
gather, so  feat @ w0 = (vision_flat @ w0_v)[idx] + grd_tokens @ w0_lm,
with the tiny [8, 1024] P matrix computed on host and gathered on-device via
an exact fp32r one-hot matmul.

All heavy matmuls are fp8 with perf_mode=DoubleRow (contraction 256 per
instruction, half-rate per output column):
  - Layer 0 (grd @ w0_lm): both operands naive e4m3 (the exact fp32 vision
    path keeps the total error at ~1.7%).
  - Layers 1-3: weights and activations split hi(e4m3) + lo(e5m2) at the
    SAME scale, z = h_hi @ (w_hi + w_lo) + h_lo @ w_hi: three half-rate
    matmuls replace two bf16 ones at near-bf16 accuracy, sharing one PSUM
    accumulation chain.
  - Layer 4 (1024->6): plain bf16 on the bf16 relu output; the 1/SW scale
    is folded into w4 so layer-3 relu needs no scale and can be split
    across the Scalar and Vector engines.
The kernel is DMA-bound (~11 MB/core at 360 GB/s): the DMA stream is
ordered in exact consumption order (grd path first, then per-layer hi
weights before lo), activations are produced in 4 pair-tiles per layer so
consumers wait at [128,2,256] granularity, and dummy fp32r matmuls keep
the PE busy ramp alive while layer 0 is DMA-paced.
"""

import ml_dtypes
import numpy as np

import concourse.bass as bass
import concourse.tile as tile
from concourse import bacc, mybir
from concourse.bass import ts
from concourse.bass_utils import run_bass_kernel_spmd

B, L, T, LM, DFF, D, H = 8, 256, 2048, 4096, 1024, 84, 4
HD = D // H
NCLS = 265
VF = D * L  # 21504 vision features per sample
NCORES = 8
TPC = T // NCORES  # 256 tokens per core
C0 = LM // 256  # 16 DoubleRow chunks for the grd matmul
C1 = DFF // 256  # 4 DoubleRow chunks for the hidden layers
JB = DFF // 128  # 8 output blocks of 128 features

S0 = 2.0 ** 12  # w0lm fp8 scale
SW = 2.0 ** 10  # w1..w3 fp8 scale

FILL_PRE = 29    # warm-up filler matmuls between one-hot and layer 0
FILL_CHUNK = 3   # fillers per layer-0 chunk (PE keeps ramp while DMA-paced)
FILL_B = 7      # fillers straddling the layer-0 -> h0-split boundary

F32 = mybir.dt.float32
F32R = mybir.dt.float32r
BF16 = mybir.dt.bfloat16
F8E4 = mybir.dt.float8e4
F8E5 = mybir.dt.float8e5
NPBF16 = ml_dtypes.bfloat16
NP8E4 = ml_dtypes.float8_e4m3
NP8E5 = ml_dtypes.float8_e5m2
RELU = mybir.ActivationFunctionType.Relu
IDENT = mybir.ActivationFunctionType.Identity
DR = mybir.MatmulPerfMode.DoubleRow
MUL = mybir.AluOpType.mult
MAX = mybir.AluOpType.max
COPY = mybir.ActivationFunctionType.Copy

_CACHE = {}


def _build_bass():
    nc = bacc.Bacc(
        "TRN2", target_bir_lowering=False, debug=False, num_devices=NCORES
    )
    inp = {}
    inp["poh"] = nc.dram_tensor("poh", [B, DFF + TPC], F32, kind="ExternalInput")
    inp["grdT"] = nc.dram_tensor("grdT", [128, C0, 2, TPC], F8E4, kind="ExternalInput")
    inp["w0lm"] = nc.dram_tensor("w0lm", [128, C0, 2, DFF], F8E4, kind="ExternalInput")
    for w in ("w1", "w2", "w3"):
        inp[w] = nc.dram_tensor(w, [128, 2, C1, 2, DFF], F8E4, kind="ExternalInput")
    inp["w4"] = nc.dram_tensor("w4", [128, JB, 6], BF16, kind="ExternalInput")
    # biasrow: [b1, b2, b3]*SW as row vectors for in-chain bias matmuls
    inp["biasrow"] = nc.dram_tensor("biasrow", [B, 3 * DFF], F32, kind="ExternalInput")
    inp["b4"] = nc.dram_tensor("b4", [6, 1], F32, kind="ExternalInput")
    out = nc.dram_tensor("out", [6, TPC], F32, kind="ExternalOutput")

    with tile.TileContext(nc) as tc:
        with (
            tc.tile_pool(name="big", bufs=1) as big,
            tc.tile_pool(name="act", bufs=2) as actp,
            tc.tile_pool(name="psum", bufs=8, space="PSUM") as pp,
            tc.tile_pool(name="outp", bufs=1) as outp,
        ):
            # --- static tiles -----------------------------------------------
            poh_sb = big.tile([B, DFF + TPC], F32R)
            paug_sb = poh_sb[:, :DFF]
            oh_sb = poh_sb[:, DFF:]
            grdT_sb = big.tile([128, C0, 2, TPC], F8E4)
            w0_sb = big.tile([128, C0, 2, DFF], F8E4)
            w_sb = {}
            for w in ("w1", "w2", "w3"):
                w_sb[w] = big.tile(
                    [128, 2, C1, 2, DFF], F8E4, name=f"{w}_sb", tag=f"{w}_sb"
                )
            w4_sb = big.tile([128, JB, 6], BF16)
            biasrow_sb = big.tile([B, 3 * DFF], F32R)
            b4_sb = big.tile([6, 1], F32)

            # --- DMA stream (issue order == consumption order) --------------
            nc.sync.dma_start(poh_sb[:], inp["poh"][:].bitcast(F32R))
            nc.sync.dma_start(grdT_sb[:], inp["grdT"][:])
            nc.sync.dma_start(biasrow_sb[:], inp["biasrow"][:].bitcast(F32R))
            for g in range(C0):
                nc.sync.dma_start(w0_sb[:, g : g + 1], inp["w0lm"][:, g : g + 1])
            for c in range(C1):
                nc.sync.dma_start(
                    w_sb["w1"][:, 0, c : c + 1], inp["w1"][:, 0, c : c + 1]
                )
            nc.sync.dma_start(w_sb["w1"][:, 1], inp["w1"][:, 1])
            nc.sync.dma_start(w_sb["w2"][:, 0], inp["w2"][:, 0])
            nc.sync.dma_start(w_sb["w2"][:, 1], inp["w2"][:, 1])
            # w3 streams last: hi, then lo in two pieces so only 8 small
            # matmuls trail the final bytes.
            nc.sync.dma_start(w_sb["w3"][:, 0], inp["w3"][:, 0])
            nc.sync.dma_start(w_sb["w3"][:, 1, 0:3], inp["w3"][:, 1, 0:3])
            nc.sync.dma_start(w_sb["w3"][:, 1, 3:4], inp["w3"][:, 1, 3:4])
            nc.sync.dma_start(b4_sb[:], inp["b4"][:])
            nc.sync.dma_start(w4_sb[:], inp["w4"][:])

            def whi(w, c, jb):
                return w_sb[w][:, 0, c, :, ts(jb, 128)]

            def wlo(w, c, jb):
                return w_sb[w][:, 1, c, :, ts(jb, 128)].bitcast(F8E5)

            # Zero stationary operand for warm-up fillers: accumulating
            # 0 @ x into a live chain is a numeric no-op that keeps PE busy.
            zt = big.tile([B, 128], F32)
            nc.vector.memset(zt[:], 0.0)

            # --- layer 0: z0 = P_pick + grdT.T @ w0lm (both fp8 e4m3) -------
            ps = [
                pp.tile([128, TPC], F32, tag="ps", name=f"ps0_{j}") for j in range(JB)
            ]

            def fill(n):
                # 64-col fp32r dummies (~107ns at any p-state) accumulated as
                # zeros into chain 7; keeps the PE ramp alive while layer 0
                # is DMA-paced.
                for i in range(n):
                    nc.tensor.matmul(
                        ps[7][:, 0:64],
                        lhsT=zt[:].bitcast(F32R),
                        rhs=oh_sb[:, 0:64],
                        start=False,
                        stop=False,
                    )
            for jb in range(JB):
                nc.tensor.matmul(
                    ps[jb][:],
                    lhsT=paug_sb[:, ts(jb, 128)],
                    rhs=oh_sb[:],
                    start=True,
                    stop=False,
                )
            fill(FILL_PRE)
            for c in range(C0):
                last = c == C0 - 1
                for jb in range(JB):
                    if last and jb == 7:
                        fill(FILL_B)
                    nc.tensor.matmul(
                        ps[jb][:],
                        lhsT=w0_sb[:, c, :, ts(jb, 128)],
                        rhs=grdT_sb[:, c],
                        start=False,
                        stop=last,
                        perf_mode=DR,
                    )
                if c < 14:
                    fill(FILL_CHUNK)

            def htiles(layer):
                t = [actp.tile([128, 2, TPC], BF16, tag=f"t{p}", name=f"t{layer}_{p}")
                     for p in range(4)]
                hhi = [actp.tile([128, 2, TPC], F8E4, tag=f"hhi{p}",
                                 name=f"hhi{layer}_{p}") for p in range(4)]
                hlo = [actp.tile([128, 2, TPC], F8E5, tag=f"hlo{p}",
                                 name=f"hlo{layer}_{p}") for p in range(4)]
                return t, hhi, hlo

            def boundary(ps, t, hhi, hlo, scale, early_slots=False):
                """psum -> hhi(e4m3) + t(bf16) + hlo(e5m2 = t - hhi).
                hhi comes straight from PSUM (no copy); work is split so the
                hhi pairs the next layer consumes first are ready earliest:
                ACT: hhi even blocks, then t odd; DVE: hhi odd, t even, subs."""
                with tc.high_priority():
                    for p in range(4):
                        nc.scalar.activation(
                            hhi[p][:, 0], ps[2 * p][:], RELU, scale=scale)
                    for p in range(4):
                        nc.vector.tensor_scalar(
                            hhi[p][:, 1], ps[2 * p + 1][:], scale, 0.0, MUL, MAX)
                if early_slots:
                    # free PSUM slots 0-3 fast so the next layer's bias
                    # starters can claim them during the w-hi DMA wait
                    for p in (0, 1):
                        nc.vector.tensor_scalar(
                            t[p][:, 0], ps[2 * p][:], scale, 0.0, MUL, MAX)
                        nc.scalar.activation(
                            t[p][:, 1], ps[2 * p + 1][:], RELU, scale=scale)
                    for p in (2, 3):
                        nc.scalar.activation(
                            t[p][:, 1], ps[2 * p + 1][:], RELU, scale=scale)
                        nc.vector.tensor_scalar(
                            t[p][:, 0], ps[2 * p][:], scale, 0.0, MUL, MAX)
                else:
                    for p in range(4):
                        nc.scalar.activation(
                            t[p][:, 1], ps[2 * p + 1][:], RELU, scale=scale)
                    for p in range(4):
                        nc.vector.tensor_scalar(
                            t[p][:, 0], ps[2 * p][:], scale, 0.0, MUL, MAX)
                for jb in range(JB):
                    nc.vector.tensor_sub(
                        hlo[jb // 2][:, jb % 2], t[jb // 2][:, jb % 2],
                        hhi[jb // 2][:, jb % 2],
                    )

            t0, h0hi, h0lo = htiles(0)
            boundary(ps, t0, h0hi, h0lo, 1.0 / S0, early_slots=True)

            # --- layers 1..3: z = h_hi@(w_hi+w_lo) + h_lo@w_hi + b ----------
            # hi-term groups c-major (consume h pairs as they appear); the
            # lo-weight group jb-major so chains stop staggered; chain ends
            # with the fp32r bias matmul (b*SW broadcast via ones vector).
            hhi, hlo = h0hi, h0lo
            for li, w in ((1, "w1"), (2, "w2"), (3, "w3")):
                ps = [
                    pp.tile([128, TPC], F32, tag="ps", name=f"ps{li}_{j}")
                    for j in range(JB)
                ]
                # chains start with the bias matmuls: no h/w dependency, so
                # they run during the w-hi DMA wait.
                for jb in range(JB):
                    nc.tensor.matmul(
                        ps[jb][:],
                        lhsT=biasrow_sb[:, ts(8 * (li - 1) + jb, 128)],
                        rhs=oh_sb[:],
                        start=True, stop=False,
                    )
                if li == 1:
                    # cover the tail of the w1-hi DMA wait
                    for i in range(4):
                        nc.tensor.matmul(
                            ps[7][:, 0:64], lhsT=zt[:].bitcast(F32R),
                            rhs=oh_sb[:, 0:64], start=False, stop=False,
                        )
                for c in range(C1):
                    for jb in range(JB):
                        nc.tensor.matmul(
                            ps[jb][:], lhsT=whi(w, c, jb), rhs=hhi[c][:],
                            start=False, stop=False, perf_mode=DR,
                        )
                # per-jb subchains: hi(lo-act) then lo-weight terms; chain
                # stops stagger across ~3.4us so the ACT/DVE boundary ops for
                # the next layer keep pace with the PE stream.
                for jb in range(JB):
                    for c in range(C1):
                        nc.tensor.matmul(
                            ps[jb][:], lhsT=whi(w, c, jb), rhs=hlo[c][:],
                            start=False, stop=False, perf_mode=DR,
                        )
                    for c in range(C1):
                        nc.tensor.matmul(
                            ps[jb][:], lhsT=wlo(w, c, jb), rhs=hhi[c][:],
                            start=False, stop=(c == C1 - 1), perf_mode=DR,
                        )
                if li < 3:
                    t, nhi, nlo = htiles(li)
                    boundary(ps, t, nhi, nlo, 1.0 / SW)
                    hhi, hlo = nhi, nlo

            # --- layer 3 relu (scale folded into w4) + layer 4 --------------
            t3 = [actp.tile([128, 2, TPC], BF16, tag=f"t{p}", name=f"t3_{p}")
                  for p in range(4)]
            ps4 = pp.tile([128, TPC], F32, tag="ps", name="ps4")[:6]
            for jb in range(JB):
                dst = t3[jb // 2][:, jb % 2]
                if jb % 2 == 0:
                    nc.scalar.activation(dst, ps[jb][:], RELU)
                else:
                    nc.vector.tensor_scalar_max(dst, ps[jb][:], 0.0)
                nc.tensor.matmul(
                    ps4[:], lhsT=w4_sb[:, jb], rhs=dst,
                    start=(jb == 0), stop=(jb == JB - 1),
                )
            out_sb = outp.tile([6, TPC], F32)
            nc.scalar.activation(out_sb[:], ps4[:], IDENT, bias=b4_sb[:, 0:1])
            nc.sync.dma_start(out[:], out_sb[:])

    nc.compile()
    return nc


def _layernorm(x, s, b):
    m = x.mean(-1, keepdims=True)
    v = ((x - m) ** 2).mean(-1, keepdims=True)
    return (x - m) / np.sqrt(v + np.float32(1e-5)) * s + b


def _host_encoder(vision_features, gauss_B, class_emb, w_in, b_in, w_out, b_out,
                  ln1_s, ln1_b, w_ff1, b_ff1, w_ff2, b_ff2, ln2_s, ln2_b):
    """Numpy fp32 replica of the reference's tiny 2-layer encoder (~2% of FLOPs)."""
    two_pi = np.float32(2.0 * np.pi)

    def fourier(xyz):
        proj = two_pi * (xyz @ gauss_B)
        return np.concatenate([np.sin(proj), np.cos(proj)], axis=-1)

    cls = vision_features[:, :, -1].astype(np.int32)
    cls = np.clip(cls, 0, NCLS - 1)  # match jax's clamped gather
    src = np.concatenate(
        [fourier(vision_features[:, :, 0:3]),
         fourier(vision_features[:, :, 3:6]),
         class_emb[cls]],
        axis=-1,
    ).astype(np.float32)  # [B, L, 84]
    pad = np.all(vision_features == 0, axis=-1)
    neg = np.where(pad, np.float32(-1e9), np.float32(0.0))[:, None, None, :]
    inv_sqrt_hd = np.float32(1.0 / np.sqrt(HD))
    for lyr in range(2):
        qkv = src @ w_in[lyr] + b_in[lyr]
        q, k, v = np.split(qkv, 3, axis=-1)
        q = q.reshape(B, L, H, HD)
        k = k.reshape(B, L, H, HD)
        v = v.reshape(B, L, H, HD)
        scores = np.einsum("blhd,bmhd->bhlm", q, k) * inv_sqrt_hd + neg
        scores = scores - scores.max(-1, keepdims=True)
        e = np.exp(scores)
        attn = e / e.sum(-1, keepdims=True)
        o = np.einsum("bhlm,bmhd->blhd", attn, v).reshape(B, L, D)
        src = _layernorm(src + o @ w_out[lyr] + b_out[lyr], ln1_s[lyr], ln1_b[lyr])
        ff = np.maximum(src @ w_ff1[lyr] + b_ff1[lyr], 0) @ w_ff2[lyr] + b_ff2[lyr]
        src = _layernorm(src + ff, ln2_s[lyr], ln2_b[lyr])
    return src.reshape(B, L * D)  # [8, 21504]


def _q8(x):
    """fp8 e4m3 (TRN variant; clip to the +-240 finite range)."""
    return np.clip(x, -240.0, 240.0).astype(NP8E4)


def _q5(x):
    return np.clip(x, -57344.0, 57344.0).astype(NP8E5)


def _dr_arrange(x, nchunk, width):
    """[256*nchunk, width] -> [128, nchunk, 2, width] DoubleRow layout."""
    return np.ascontiguousarray(
        x.reshape(nchunk, 2, 128, width).transpose(2, 0, 1, 3)
    )


def kernel(grd_tokens, vision_features, token_batch_idx, gauss_B, class_emb,
           w_in, b_in, w_out, b_out, ln1_s, ln1_b, w_ff1, b_ff1, w_ff2, b_ff2,
           ln2_s, ln2_b, w0, b0, w1, b1, w2, b2, w3, b3, w4, b4,
           _trace=False):
    f32 = np.float32
    grd_tokens = np.asarray(grd_tokens, f32)
    vision_features = np.asarray(vision_features, f32)
    idx = np.asarray(token_batch_idx).astype(np.int64)
    w0 = np.asarray(w0, f32)
    b0 = np.asarray(b0, f32)

    # Vision branch on host (input marshalling + ~2.3 GF): encoder -> P matrix.
    vision_flat = _host_encoder(
        vision_features, np.asarray(gauss_B, f32), np.asarray(class_emb, f32),
        np.asarray(w_in, f32), np.asarray(b_in, f32), np.asarray(w_out, f32),
        np.asarray(b_out, f32), np.asarray(ln1_s, f32), np.asarray(ln1_b, f32),
        np.asarray(w_ff1, f32), np.asarray(b_ff1, f32), np.asarray(w_ff2, f32),
        np.asarray(b_ff2, f32), np.asarray(ln2_s, f32), np.asarray(ln2_b, f32),
    )
    paug = ((vision_flat @ w0[:VF] + b0) * f32(S0)).astype(f32)  # [8, 1024]

    # Shared (replicated) device inputs.
    shared = {"w0lm": _dr_arrange(_q8(w0[VF:] * f32(S0)), C0, DFF)}
    for name, w in (("w1", w1), ("w2", w2), ("w3", w3)):
        ws = np.asarray(w, f32) * f32(SW)
        hi = _q8(ws)
        lo = _q5(ws - hi.astype(f32))
        shared[name] = np.ascontiguousarray(
            np.stack(
                [
                    _dr_arrange(hi, C1, DFF),
                    _dr_arrange(lo.view(np.uint8).view(NP8E4), C1, DFF),
                ],
                axis=1,
            )
        )  # [128, 2, C1, 2, DFF] e4m3 container
    shared["w4"] = np.ascontiguousarray(
        (np.asarray(w4, f32) * f32(1.0 / SW))
        .reshape(JB, 128, 6).transpose(1, 0, 2).astype(NPBF16)
    )
    brow = np.concatenate([np.asarray(b, f32) for b in (b1, b2, b3)]) * f32(SW)
    shared["biasrow"] = np.ascontiguousarray(np.broadcast_to(brow, (B, 3 * DFF)))
    shared["b4"] = np.ascontiguousarray(np.asarray(b4, f32).reshape(6, 1))

    # Per-core shards.
    in_maps = []
    for m in range(NCORES):
        rows = slice(m * TPC, (m + 1) * TPC)
        grdT = _dr_arrange(_q8(grd_tokens[rows].T), C0, TPC)
        oh = (idx[rows][None, :] == np.arange(B)[:, None]).astype(f32)
        im = dict(shared)
        im["grdT"] = grdT
        im["poh"] = np.ascontiguousarray(np.concatenate([paug, oh], axis=1))
        in_maps.append(im)

    if "nc" not in _CACHE:
        _CACHE["nc"] = _build_bass()
    res = run_bass_kernel_spmd(
        _CACHE["nc"], in_maps, core_ids=list(range(NCORES)), trace=_trace
    )
    _CACHE["last_result"] = res
    out = np.concatenate([r["out"].T for r in res.results], axis=0)
    return np.ascontiguousarray(out.astype(f32))
